# revision 89
# baseline (speedup 1.0000x reference)
"""DiceLoss kernel for Trainium2, data-parallel over 8 NeuronCores.

Algorithm (per core, 2 of 16 batches), bf16 pipeline:
  - Host converts x to bf16 (round-to-nearest) and target to uint8 (both
    pure dtype conversions, mirroring the original int64->u8 cast). Each
    core DMAs its x slice as ONE fused [128, 4, fd] class-major tile per
    segment plus a [128, fd] u8 target tile (half the HBM bytes of f32).
  - argmax one-hot lanes E = (e0, e1, e2, 1) with e_c = [x_c == mx]:
    mx via a fused plane-pair TT max + a reducing TT max; e-lanes via
    TT is_equal against a stride-0 broadcast AP of mx — two or three
    lanes per instruction. All packed-bf16 on DVE (2x_1p fast path),
    except some segments' e2 which runs on Pool as subtract + tensor
    scalar is_equal (Pool supports no TT compare / STT) for balance.
  - target MOMENT lanes T = (1, t, t^2, |t-1|) from the uint8 labels,
    class-MINOR [128, 4f+j], written by ACT (copy/Square/Abs, whose cost
    is stride-insensitive) and DVE TensorCopy for a couple of segments.
  - E stays class-MAJOR [128, 4, fd] (packed writes); the TensorEngine
    uses T chunks as the single-free-dim stationary weights and streams
    E through a multi-dim (f outer, c inner) AP, accumulating
    O[4f'+j, 4f+c] += sum_p T_j * E_c in PSUM across all chunks. The
    4x4 diagonal blocks of O sum to M'[c, j] = sum_pix e_c * mu_j(t).
  - Host sums the 8 per-core [128,128] PSUM dumps, inverts the 4x4 moment
    basis (exact integers) to get the confusion matrix, and finishes the
    (2i+eps)/(u+eps) division and the mean in f32 like the reference.

All sums are integer-valued f32 < 2^24 so device arithmetic is exact; the
only deviation from the f32 reference is the bf16 rounding of x before
the argmax (ties/flips on a ~0.2% pixel tail, measured rel err ~2e-5,
far inside the 2e-2 tolerance).

Schedule notes (TimelineSim-guided): 7 pixel segments sized so the first
transfer lands early and the final segment's lane->matmul->PSUM-dump tail
is short; the last segment's x DMA and lane ops are split into halves so
its matmuls overlap its own transfer tail; constant-lane memsets ride
DVE's idle window before the first transfer; per-segment e2 / t-copy
engine placement balances DVE/ACT/Pool just under the ~13.5us DMA
roofline (4.5 MiB/core at 360 B/ns).
"""
import sys

sys.path.insert(0, "/opt/trn_rl_repo")

import numpy as np

B, C, H, W = 16, 4, 512, 512
N_CORES = 8
B_LOC = B // N_CORES          # 2 batches per core
EPS = 1e-6
P = 128                       # SBUF partitions
FDMAX = 1024                  # max free-dim of one pixel tile
PLANE = H * W                 # 262144 pixels per (b, c) plane

# Pixel segments per core: (batch, flat_start, fd). Each covers pixels
# [start, start + 128*fd) of that batch's plane; partition k owns
# [start + k*fd, start + (k+1)*fd). First segment is small so compute
# starts early; the last is small so the post-last-DMA tail is short.
SEGS = [
    (0, 0, 608),
    (0, 608 * P, 704),
    (0, (608 + 704) * P, 736),
    (1, 0, 704),
    (1, 704 * P, 288),
    (1, (704 + 288) * P, 736),
    (1, (704 + 288 + 736) * P, 320),
]
assert sum(P * fd for b, s, fd in SEGS) == B_LOC * PLANE
assert all(fd % 32 == 0 for _, _, fd in SEGS)
NT = len(SEGS)
NCH_TOT = sum(4 * fd // 128 for _, _, fd in SEGS)
# DMA issue order (sequential: HWDGE descriptor generation is a serial
# 625ns/DMA pipeline, so reordering only starves the transfer engine).
DMA_ORDER = list(range(NT))
# lane1 (copy of t) placement: DVE TensorCopy for these segment ids
# (2x_2p fast path), ACT copy otherwise
COPY_ON_DVE = {4}
# e2 placement: DVE (fused eq-tri) for these segments, Pool (sub + eq,
# two ops) otherwise. Pool's two-op form is expensive, so it only covers
# the middle segments; first/last stay on DVE whose queue is free there.
E2_ON_DVE = {0, 4, NT - 2, NT - 1}
assert COPY_ON_DVE <= set(range(NT)) and E2_ON_DVE <= set(range(NT))
# issue t DMAs from the Pool queue (SWDGE) so their descriptor generation
# doesn't serialize with the x DMAs on the shared HWDGE
T_ON_POOL = False
# E/T pipeline depth (segment it uses buffer it % N_EBUF)
N_EBUF = 3
# segments whose x DMA and DVE lane ops are split into two halves so the
# final lanes/matmuls overlap the tail of their own transfer
HALF_SPLIT = {NT - 1}
# split each x DMA into class pairs (planes 0-1 then 2-3) with a same-pair
# max tree. Measured worse than the fused DMA for every segment set (the
# extra HWDGE descriptor generations delay later arrivals more than the
# earlier m01 start saves), so disabled.
SPLIT_X_PAIRS = set()
# T lane3 mode: "abs" = |t-1| on ACT (basis row [1,0,1,2]);
# "eq2" = [t == 2] via one-op tensor_scalar, per-seg DVE/Pool placement
# (basis row [0,0,1,0]). The basis must be globally consistent because
# PSUM accumulates across segments.
LANE3_MODE = "abs"
LANE3_ON_DVE = set()
# issue each segment's t DMA after the NEXT segment's x DMA
T_DELAY = False
# work-pool (mp/mx/d2) buffer depth
WORK_BUFS = 6
# how many segments ahead of compute to issue DMAs (NT = all upfront)
DMA_LOOKAHEAD = 2


def build_body(tc, outs, ins, n_reps=1):
    """Kernel body. ins = {"x": AP [B_LOC,C,H,W] bf16, "t": AP [B_LOC,H,W] u8}
    outs = {"conf": AP [128,128] f32}. n_reps>1 repeats the whole pass
    (PSUM keeps accumulating; used for timing-by-differencing)."""
    import concourse.mybir as mybir

    nc = tc.nc
    f32 = mybir.dt.float32
    bf16 = mybir.dt.bfloat16
    AF = mybir.ActivationFunctionType
    OP = mybir.AluOpType

    x = ins["x"]
    t = ins["t"]
    conf = outs["conf"]

    xf = x.rearrange("b c h w -> b c (h w)")
    tfl = t.rearrange("b h w -> b (h w)")

    NEB = N_EBUF  # E/T buffer count
    with (
        tc.tile_pool(name="xin", bufs=1) as xin,
        tc.tile_pool(name="tin", bufs=1) as tin,
        tc.tile_pool(name="work", bufs=WORK_BUFS) as work,
        tc.tile_pool(name="eht", bufs=1) as eht,
        tc.tile_pool(name="psum", bufs=1, space="PSUM") as psum,
    ):
        P_acc = psum.tile([P, 128], f32, name="P_acc")
        bias_m1 = None
        if LANE3_MODE == "abs":
            bias_m1 = eht.tile([P, 1], f32, name="bias_m1")
            nc.gpsimd.memset(bias_m1, -1.0)
        # E is class-MAJOR [p, c, f] so the eq lanes are packed-bf16 writes
        # (DVE 2x); the matmul reads it as the MOVING operand through a
        # multi-dim (f, c) AP. T is class-MINOR [p, 4f+j] because the PE
        # weights operand must be a single free dim; its lanes are written
        # by ACT/Pool whose cost is stride-insensitive.
        Es = [eht.tile([P, 4, FDMAX], bf16, name=f"Ebuf{i}") for i in range(NEB)]
        Ts = [eht.tile([P, 4 * FDMAX], bf16, name=f"Tbuf{i}") for i in range(NEB)]

        n_mm = n_reps * NCH_TOT
        mm = 0
        for rep in range(n_reps):
            # Dedicated tiles per segment; DMA issue runs DMA_LOOKAHEAD
            # segments ahead of compute.
            xts = [None] * NT
            tus = [None] * NT
            issued = [0]

            def issue_t(it):
                b_i, seg_start, fd = SEGS[it]
                npix = P * fd
                tu = tin.tile([P, FDMAX], mybir.dt.uint8, name=f"tu{it}")[
                    :, :fd
                ]
                t_eng = nc.gpsimd if T_ON_POOL else nc.sync
                t_eng.dma_start(
                    out=tu,
                    in_=tfl[b_i, seg_start : seg_start + npix].rearrange(
                        "(p f) -> p f", f=fd
                    ),
                )
                tus[it] = tu

            def issue_dmas(upto):
                while issued[0] <= min(upto, NT - 1):
                    i = issued[0]
                    it = DMA_ORDER[i]
                    issued[0] += 1
                    b_i, seg_start, fd = SEGS[it]
                    npix = P * fd
                    xt = xin.tile([P, 4, FDMAX], bf16, name=f"xt{it}")[:, :, :fd]
                    x_src = xf[b_i, :, seg_start : seg_start + npix].rearrange(
                        "c (p f) -> p c f", f=fd
                    )
                    if it in HALF_SPLIT:
                        h = fd // 2
                        nc.sync.dma_start(out=xt[:, :, :h], in_=x_src[:, :, :h])
                        nc.sync.dma_start(out=xt[:, :, h:], in_=x_src[:, :, h:])
                    elif it in SPLIT_X_PAIRS:
                        nc.sync.dma_start(out=xt[:, 0:2], in_=x_src[:, 0:2])
                        nc.sync.dma_start(out=xt[:, 2:4], in_=x_src[:, 2:4])
                    else:
                        nc.sync.dma_start(out=xt, in_=x_src)
                    xts[it] = xt
                    if T_DELAY:
                        if i > 0:
                            issue_t(DMA_ORDER[i - 1])
                        if i == NT - 1:
                            issue_t(DMA_ORDER[i])
                    if not T_DELAY:
                        issue_t(it)
                    # constant lanes (E lane 3 = 1 collects target moments,
                    # T lane 0 = 1 collects pred counts), hidden under the
                    # first transfers.
                    if rep == 0 and it < NEB:
                        # DVE idles until the first transfer lands, so most
                        # constant-lane memsets ride in that window for free.
                        T4i = Ts[it].rearrange("p (f c) -> p f c", c=4)
                        (nc.vector if it < 2 else nc.gpsimd).memset(
                            Es[it][:, 3, :], 1.0
                        )
                        (nc.vector if it == 0 else nc.gpsimd).memset(
                            T4i[:, :, 0], 1.0
                        )

            for it in range(NT):
                issue_dmas(it + DMA_LOOKAHEAD)
                b_i, seg_start, fd = SEGS[it]
                xt = xts[it]
                tu = tus[it]

                # max over the 4 class planes, fused as plane-pairs: one TT
                # computes (max(x0,x2), max(x1,x3)), a second reduces the
                # pair. All packed-bf16 on DVE (2x_1p). HALF_SPLIT segments
                # run the lane block per half so the final lanes/matmuls
                # overlap their own transfer tail.
                mp = work.tile([P, 2, FDMAX], bf16, name="mp")[:, :, :fd]
                mx = work.tile([P, FDMAX], bf16, name="mx")[:, :fd]

                E = Es[it % NEB]
                T = Ts[it % NEB]

                def bcast(ap, n):
                    return type(ap)(ap.tensor, ap.offset,
                                    [ap.ap[0], [0, n]] + ap.ap[1:])

                halves = (
                    [(0, fd // 2), (fd // 2, fd)]
                    if it in HALF_SPLIT
                    else [(0, fd)]
                )
                for a, b in halves:
                    mph = mp[:, :, a:b]
                    mxh = mx[:, a:b]
                    if it in SPLIT_X_PAIRS and it not in HALF_SPLIT:
                        # same-pair tree: m01 depends only on the first
                        # class-pair transfer, m23 on the second
                        nc.vector.tensor_tensor(
                            mph[:, 0], xt[:, 0, a:b], xt[:, 1, a:b], OP.max
                        )
                        nc.vector.tensor_tensor(
                            mph[:, 1], xt[:, 2, a:b], xt[:, 3, a:b], OP.max
                        )
                    else:
                        nc.vector.tensor_tensor(
                            mph, xt[:, 0:2, a:b], xt[:, 2:4, a:b], OP.max
                        )
                    nc.vector.tensor_tensor(mxh, mph[:, 0], mph[:, 1], OP.max)

                    # pred one-hot lanes 0..2 (lane 3 stays 1.0). e0/e1
                    # fused in one DVE TT against a stride-0 broadcast of
                    # mx; e2 on Pool except where Pool's latency would gate
                    # the final matmuls.
                    if it in E2_ON_DVE:
                        nc.vector.tensor_tensor(
                            E[:, 0:3, a:b], xt[:, 0:3, a:b], bcast(mxh, 3),
                            OP.is_equal,
                        )
                    else:
                        nc.vector.tensor_tensor(
                            E[:, 0:2, a:b], xt[:, 0:2, a:b], bcast(mxh, 2),
                            OP.is_equal,
                        )
                        # Pool has no STT / TT is_equal: subtract (exactly 0
                        # iff x2 == mx in bf16) then compare against 0.
                        d2 = work.tile([P, FDMAX], bf16, name="d2")[:, a:b]
                        nc.gpsimd.tensor_tensor(
                            d2, xt[:, 2, a:b], mxh, OP.subtract
                        )
                        nc.gpsimd.tensor_scalar(
                            E[:, 2, a:b], d2, 0.0, None, OP.is_equal
                        )

                # target MOMENT lanes (class-minor), from the uint8 labels:
                #   lane 0 = 1 (memset), lane 1 = t (ACT copy, or DVE
                #   TensorCopy for balance), lane 2 = t^2 (ACT Square),
                #   lane 3 = [t == 2] (one-op tensor_scalar on Pool/DVE)
                T4 = T[:, : 4 * fd].rearrange("p (f c) -> p f c", c=4)
                if it in COPY_ON_DVE:
                    nc.vector.tensor_copy(T4[:, :, 1], tu)
                else:
                    nc.scalar.copy(T4[:, :, 1], tu)
                nc.scalar.activation(T4[:, :, 2], tu, AF.Square)
                if LANE3_MODE == "abs":
                    nc.scalar.activation(
                        T4[:, :, 3], tu, AF.Abs, bias=bias_m1, scale=1.0
                    )
                else:
                    lane3_eng = nc.vector if it in LANE3_ON_DVE else nc.gpsimd
                    lane3_eng.tensor_scalar(
                        T4[:, :, 3], tu, 2.0, None, OP.is_equal
                    )

                # 128-wide chunks: T (class-minor, contiguous) is the
                # stationary weights; E streams as the moving operand via a
                # multi-dim (f outer, c inner) AP. PSUM entry [4f'+j, 4f+c]
                # accumulates sum_p T_j * E_c; the host decodes the 4x4
                # diagonal blocks (transposed vs the E^T @ T layout).
                for w_i in range(fd // 32):
                    sl = slice(w_i * 32, (w_i + 1) * 32)
                    Ec = E[:, :, sl].rearrange("p c f -> p f c")
                    Tc = T[:, 128 * w_i : 128 * (w_i + 1)]
                    nc.tensor.matmul(
                        P_acc, Tc, Ec,
                        start=(mm == 0), stop=(mm == n_mm - 1),
                    )
                    mm += 1

        conf_sb = eht.tile([P, 128], f32, name="conf_sb")
        nc.vector.tensor_copy(conf_sb, P_acc)
        nc.sync.dma_start(out=conf, in_=conf_sb)


_NC_CACHE = {}


def _get_nc(n_reps=1):
    if n_reps in _NC_CACHE:
        return _NC_CACHE[n_reps]
    import concourse.bacc as bacc
    import concourse.mybir as mybir
    import concourse.tile as tile

    nc = bacc.Bacc(
        "TRN2",
        target_bir_lowering=False,
        debug=False,
        enable_asserts=False,
        num_devices=N_CORES,
    )
    x = nc.dram_tensor(
        "x", [B_LOC, C, H, W], mybir.dt.bfloat16, kind="ExternalInput"
    ).ap()
    t = nc.dram_tensor("t", [B_LOC, H, W], mybir.dt.uint8, kind="ExternalInput").ap()
    conf = nc.dram_tensor("conf", [P, 128], mybir.dt.float32, kind="ExternalOutput").ap()

    with tile.TileContext(nc) as tc:
        build_body(tc, {"conf": conf}, {"x": x, "t": t}, n_reps=n_reps)
    nc.compile()
    _NC_CACHE[n_reps] = nc
    return nc


# Moment basis: T-lane j holds mu_j(t); V[j, d] = mu_j(d) for class d.
# Row 3 matches LANE3_MODE ("abs" -> |t-1|, "eq2" -> [t==2]).
def _mom_v() -> np.ndarray:
    row3 = [1, 0, 1, 2] if LANE3_MODE == "abs" else [0, 0, 1, 0]
    return np.array(
        [[1, 1, 1, 1], [0, 1, 2, 3], [0, 1, 4, 9], row3], dtype=np.float64
    )


def make_in_maps(x_full: np.ndarray, t_full: np.ndarray) -> list:
    """Shard full inputs into per-core {x: bf16, t: u8} maps."""
    import ml_dtypes

    x_bf = np.asarray(x_full, dtype=np.float32).astype(ml_dtypes.bfloat16)
    t_u8 = np.asarray(t_full).astype(np.uint8)
    in_maps = []
    for ci in range(N_CORES):
        sl = slice(ci * B_LOC, (ci + 1) * B_LOC)
        in_maps.append(
            {
                "x": np.ascontiguousarray(x_bf[sl]),
                "t": np.ascontiguousarray(t_u8[sl]),
            }
        )
    return in_maps


def decode_conf(conf_sum: np.ndarray) -> np.ndarray:
    """[128,128] summed PSUM dump(s) -> moment-basis matrix M' [4,4].

    M'[c, j] = sum_pix elane_c * mu_j(t), with elane = (e0, e1, e2, 1).
    PSUM holds O[4i+j, 4i+c] (T as weights, E as moving operand), so the
    diagonal-block sum is transposed into [c, j]."""
    O = conf_sum.reshape(32, 4, 32, 4)
    return O[np.arange(32), :, np.arange(32), :].sum(axis=0).T


def finish(Mp: np.ndarray) -> np.float32:
    """Moment-basis M' [4,4] -> dice loss scalar (f32 math as the reference)."""
    mom_v = _mom_v()
    Mp = Mp.astype(np.float64)
    # rows c<3: M[c, :] (target-class histogram within pred class c)
    M_rows = np.linalg.solve(mom_v, Mp[:3, :].T).T  # [3, 4]
    M_rows = np.rint(M_rows)
    tgt = np.rint(np.linalg.solve(mom_v, Mp[3, :]))  # [4]
    n_tot = Mp[3, 0]
    pred = np.empty(4)
    pred[:3] = Mp[:3, 0]
    pred[3] = n_tot - pred[:3].sum()
    inter = np.empty(4)
    inter[:3] = np.diag(M_rows[:, :3])
    inter[3] = tgt[3] - M_rows[:, 3].sum()

    inter32 = inter.astype(np.float32)
    union32 = (pred + tgt).astype(np.float32)
    eps32 = np.float32(EPS)
    dice = (np.float32(2.0) * inter32 + eps32) / (union32 + eps32)
    losses = np.float32(1.0) - dice
    return np.float32(losses.mean(dtype=np.float32))


LAST_RESULT = None


def kernel(**inputs) -> np.ndarray:
    from concourse import bass_utils

    nc = _get_nc()
    in_maps = make_in_maps(inputs["input"], inputs["target"])

    # Transient NRT device errors (e.g. NRT_EXEC_UNIT_UNRECOVERABLE) have
    # been observed to succeed on retry in this environment.
    last_exc = None
    for attempt in range(3):
        try:
            res = bass_utils.run_bass_kernel_spmd(
                nc, in_maps, core_ids=list(range(N_CORES))
            )
            break
        except Exception as exc:  # noqa: BLE001
            last_exc = exc
            import time as _time

            _time.sleep(2.0 * (attempt + 1))
    else:
        raise last_exc
    global LAST_RESULT
    LAST_RESULT = res

    conf_sum = np.zeros((P, 128), dtype=np.float64)
    for r in res.results:
        conf_sum += r["conf"].astype(np.float64)
    Mp = decode_conf(conf_sum)
    return finish(Mp)


# revision 90
# speedup vs baseline: 1.0022x; 1.0022x over previous
"""DiceLoss kernel for Trainium2, data-parallel over 8 NeuronCores.

Algorithm (per core, 2 of 16 batches), bf16 pipeline:
  - Host converts x to bf16 (round-to-nearest) and target to uint8 (both
    pure dtype conversions, mirroring the original int64->u8 cast). Each
    core DMAs its x slice as ONE fused [128, 4, fd] class-major tile per
    segment plus a [128, fd] u8 target tile (half the HBM bytes of f32).
  - argmax one-hot lanes E = (e0, e1, e2, 1) with e_c = [x_c == mx]:
    mx via a fused plane-pair TT max + a reducing TT max; e-lanes via
    TT is_equal against a stride-0 broadcast AP of mx — two or three
    lanes per instruction. All packed-bf16 on DVE (2x_1p fast path),
    except some segments' e2 which runs on Pool as subtract + tensor
    scalar is_equal (Pool supports no TT compare / STT) for balance.
  - target MOMENT lanes T = (1, t, t^2, |t-1|) from the uint8 labels,
    class-MINOR [128, 4f+j], written by ACT (copy/Square/Abs, whose cost
    is stride-insensitive) and DVE TensorCopy for a couple of segments.
  - E stays class-MAJOR [128, 4, fd] (packed writes); the TensorEngine
    uses T chunks as the single-free-dim stationary weights and streams
    E through a multi-dim (f outer, c inner) AP, accumulating
    O[4f'+j, 4f+c] += sum_p T_j * E_c in PSUM across all chunks. The
    4x4 diagonal blocks of O sum to M'[c, j] = sum_pix e_c * mu_j(t).
  - Host sums the 8 per-core [128,128] PSUM dumps, inverts the 4x4 moment
    basis (exact integers) to get the confusion matrix, and finishes the
    (2i+eps)/(u+eps) division and the mean in f32 like the reference.

All sums are integer-valued f32 < 2^24 so device arithmetic is exact; the
only deviation from the f32 reference is the bf16 rounding of x before
the argmax (ties/flips on a ~0.2% pixel tail, measured rel err ~2e-5,
far inside the 2e-2 tolerance).

Schedule notes (TimelineSim-guided): 7 pixel segments sized so the first
transfer lands early and the final segment's lane->matmul->PSUM-dump tail
is short; the last segment's x DMA and lane ops are split into halves so
its matmuls overlap its own transfer tail; constant-lane memsets ride
DVE's idle window before the first transfer; per-segment e2 / t-copy
engine placement balances DVE/ACT/Pool just under the ~13.5us DMA
roofline (4.5 MiB/core at 360 B/ns).
"""
import sys

sys.path.insert(0, "/opt/trn_rl_repo")

import numpy as np

B, C, H, W = 16, 4, 512, 512
N_CORES = 8
B_LOC = B // N_CORES          # 2 batches per core
EPS = 1e-6
P = 128                       # SBUF partitions
FDMAX = 1024                  # max free-dim of one pixel tile
PLANE = H * W                 # 262144 pixels per (b, c) plane

# Pixel segments per core: (batch, flat_start, fd). Each covers pixels
# [start, start + 128*fd) of that batch's plane; partition k owns
# [start + k*fd, start + (k+1)*fd). First segment is small so compute
# starts early; the last is small so the post-last-DMA tail is short.
SEGS = [
    (0, 0, 608),
    (0, 608 * P, 672),
    (0, (608 + 672) * P, 768),
    (1, 0, 704),
    (1, 704 * P, 288),
    (1, (704 + 288) * P, 736),
    (1, (704 + 288 + 736) * P, 320),
]
assert sum(P * fd for b, s, fd in SEGS) == B_LOC * PLANE
assert all(fd % 32 == 0 for _, _, fd in SEGS)
NT = len(SEGS)
NCH_TOT = sum(4 * fd // 128 for _, _, fd in SEGS)
# DMA issue order (sequential: HWDGE descriptor generation is a serial
# 625ns/DMA pipeline, so reordering only starves the transfer engine).
DMA_ORDER = list(range(NT))
# lane1 (copy of t) placement: DVE TensorCopy for these segment ids
# (2x_2p fast path), ACT copy otherwise
COPY_ON_DVE = {4}
# e2 placement: DVE (fused eq-tri) for these segments, Pool (sub + eq,
# two ops) otherwise. Pool's two-op form is expensive, so it only covers
# the middle segments; first/last stay on DVE whose queue is free there.
E2_ON_DVE = {0, 4, NT - 2, NT - 1}
assert COPY_ON_DVE <= set(range(NT)) and E2_ON_DVE <= set(range(NT))
# issue t DMAs from the Pool queue (SWDGE) so their descriptor generation
# doesn't serialize with the x DMAs on the shared HWDGE
T_ON_POOL = False
# E/T pipeline depth (segment it uses buffer it % N_EBUF)
N_EBUF = 3
# segments whose x DMA and DVE lane ops are split into two halves so the
# final lanes/matmuls overlap the tail of their own transfer
HALF_SPLIT = {NT - 1}
# split each x DMA into class pairs (planes 0-1 then 2-3) with a same-pair
# max tree. Measured worse than the fused DMA for every segment set (the
# extra HWDGE descriptor generations delay later arrivals more than the
# earlier m01 start saves), so disabled.
SPLIT_X_PAIRS = set()
# T lane3 mode: "abs" = |t-1| on ACT (basis row [1,0,1,2]);
# "eq2" = [t == 2] via one-op tensor_scalar, per-seg DVE/Pool placement
# (basis row [0,0,1,0]). The basis must be globally consistent because
# PSUM accumulates across segments.
LANE3_MODE = "abs"
LANE3_ON_DVE = set()
# issue each segment's t DMA after the NEXT segment's x DMA
T_DELAY = False
# work-pool (mp/mx/d2) buffer depth
WORK_BUFS = 6
# how many segments ahead of compute to issue DMAs (NT = all upfront)
DMA_LOOKAHEAD = 2


def build_body(tc, outs, ins, n_reps=1):
    """Kernel body. ins = {"x": AP [B_LOC,C,H,W] bf16, "t": AP [B_LOC,H,W] u8}
    outs = {"conf": AP [128,128] f32}. n_reps>1 repeats the whole pass
    (PSUM keeps accumulating; used for timing-by-differencing)."""
    import concourse.mybir as mybir

    nc = tc.nc
    f32 = mybir.dt.float32
    bf16 = mybir.dt.bfloat16
    AF = mybir.ActivationFunctionType
    OP = mybir.AluOpType

    x = ins["x"]
    t = ins["t"]
    conf = outs["conf"]

    xf = x.rearrange("b c h w -> b c (h w)")
    tfl = t.rearrange("b h w -> b (h w)")

    NEB = N_EBUF  # E/T buffer count
    with (
        tc.tile_pool(name="xin", bufs=1) as xin,
        tc.tile_pool(name="tin", bufs=1) as tin,
        tc.tile_pool(name="work", bufs=WORK_BUFS) as work,
        tc.tile_pool(name="eht", bufs=1) as eht,
        tc.tile_pool(name="psum", bufs=1, space="PSUM") as psum,
    ):
        P_acc = psum.tile([P, 128], f32, name="P_acc")
        bias_m1 = None
        if LANE3_MODE == "abs":
            bias_m1 = eht.tile([P, 1], f32, name="bias_m1")
            nc.gpsimd.memset(bias_m1, -1.0)
        # E is class-MAJOR [p, c, f] so the eq lanes are packed-bf16 writes
        # (DVE 2x); the matmul reads it as the MOVING operand through a
        # multi-dim (f, c) AP. T is class-MINOR [p, 4f+j] because the PE
        # weights operand must be a single free dim; its lanes are written
        # by ACT/Pool whose cost is stride-insensitive.
        Es = [eht.tile([P, 4, FDMAX], bf16, name=f"Ebuf{i}") for i in range(NEB)]
        Ts = [eht.tile([P, 4 * FDMAX], bf16, name=f"Tbuf{i}") for i in range(NEB)]

        n_mm = n_reps * NCH_TOT
        mm = 0
        for rep in range(n_reps):
            # Dedicated tiles per segment; DMA issue runs DMA_LOOKAHEAD
            # segments ahead of compute.
            xts = [None] * NT
            tus = [None] * NT
            issued = [0]

            def issue_t(it):
                b_i, seg_start, fd = SEGS[it]
                npix = P * fd
                tu = tin.tile([P, FDMAX], mybir.dt.uint8, name=f"tu{it}")[
                    :, :fd
                ]
                t_eng = nc.gpsimd if T_ON_POOL else nc.sync
                t_eng.dma_start(
                    out=tu,
                    in_=tfl[b_i, seg_start : seg_start + npix].rearrange(
                        "(p f) -> p f", f=fd
                    ),
                )
                tus[it] = tu

            def issue_dmas(upto):
                while issued[0] <= min(upto, NT - 1):
                    i = issued[0]
                    it = DMA_ORDER[i]
                    issued[0] += 1
                    b_i, seg_start, fd = SEGS[it]
                    npix = P * fd
                    xt = xin.tile([P, 4, FDMAX], bf16, name=f"xt{it}")[:, :, :fd]
                    x_src = xf[b_i, :, seg_start : seg_start + npix].rearrange(
                        "c (p f) -> p c f", f=fd
                    )
                    if it in HALF_SPLIT:
                        h = fd // 2
                        nc.sync.dma_start(out=xt[:, :, :h], in_=x_src[:, :, :h])
                        nc.sync.dma_start(out=xt[:, :, h:], in_=x_src[:, :, h:])
                    elif it in SPLIT_X_PAIRS:
                        nc.sync.dma_start(out=xt[:, 0:2], in_=x_src[:, 0:2])
                        nc.sync.dma_start(out=xt[:, 2:4], in_=x_src[:, 2:4])
                    else:
                        nc.sync.dma_start(out=xt, in_=x_src)
                    xts[it] = xt
                    if T_DELAY:
                        if i > 0:
                            issue_t(DMA_ORDER[i - 1])
                        if i == NT - 1:
                            issue_t(DMA_ORDER[i])
                    if not T_DELAY:
                        issue_t(it)
                    # constant lanes (E lane 3 = 1 collects target moments,
                    # T lane 0 = 1 collects pred counts), hidden under the
                    # first transfers.
                    if rep == 0 and it < NEB:
                        # DVE idles until the first transfer lands, so most
                        # constant-lane memsets ride in that window for free.
                        T4i = Ts[it].rearrange("p (f c) -> p f c", c=4)
                        (nc.vector if it < 2 else nc.gpsimd).memset(
                            Es[it][:, 3, :], 1.0
                        )
                        (nc.vector if it == 0 else nc.gpsimd).memset(
                            T4i[:, :, 0], 1.0
                        )

            for it in range(NT):
                issue_dmas(it + DMA_LOOKAHEAD)
                b_i, seg_start, fd = SEGS[it]
                xt = xts[it]
                tu = tus[it]

                # max over the 4 class planes, fused as plane-pairs: one TT
                # computes (max(x0,x2), max(x1,x3)), a second reduces the
                # pair. All packed-bf16 on DVE (2x_1p). HALF_SPLIT segments
                # run the lane block per half so the final lanes/matmuls
                # overlap their own transfer tail.
                mp = work.tile([P, 2, FDMAX], bf16, name="mp")[:, :, :fd]
                mx = work.tile([P, FDMAX], bf16, name="mx")[:, :fd]

                E = Es[it % NEB]
                T = Ts[it % NEB]

                def bcast(ap, n):
                    return type(ap)(ap.tensor, ap.offset,
                                    [ap.ap[0], [0, n]] + ap.ap[1:])

                halves = (
                    [(0, fd // 2), (fd // 2, fd)]
                    if it in HALF_SPLIT
                    else [(0, fd)]
                )
                for a, b in halves:
                    mph = mp[:, :, a:b]
                    mxh = mx[:, a:b]
                    if it in SPLIT_X_PAIRS and it not in HALF_SPLIT:
                        # same-pair tree: m01 depends only on the first
                        # class-pair transfer, m23 on the second
                        nc.vector.tensor_tensor(
                            mph[:, 0], xt[:, 0, a:b], xt[:, 1, a:b], OP.max
                        )
                        nc.vector.tensor_tensor(
                            mph[:, 1], xt[:, 2, a:b], xt[:, 3, a:b], OP.max
                        )
                    else:
                        nc.vector.tensor_tensor(
                            mph, xt[:, 0:2, a:b], xt[:, 2:4, a:b], OP.max
                        )
                    nc.vector.tensor_tensor(mxh, mph[:, 0], mph[:, 1], OP.max)

                    # pred one-hot lanes 0..2 (lane 3 stays 1.0). e0/e1
                    # fused in one DVE TT against a stride-0 broadcast of
                    # mx; e2 on Pool except where Pool's latency would gate
                    # the final matmuls.
                    if it in E2_ON_DVE:
                        nc.vector.tensor_tensor(
                            E[:, 0:3, a:b], xt[:, 0:3, a:b], bcast(mxh, 3),
                            OP.is_equal,
                        )
                    else:
                        nc.vector.tensor_tensor(
                            E[:, 0:2, a:b], xt[:, 0:2, a:b], bcast(mxh, 2),
                            OP.is_equal,
                        )
                        # Pool has no STT / TT is_equal: subtract (exactly 0
                        # iff x2 == mx in bf16) then compare against 0.
                        d2 = work.tile([P, FDMAX], bf16, name="d2")[:, a:b]
                        nc.gpsimd.tensor_tensor(
                            d2, xt[:, 2, a:b], mxh, OP.subtract
                        )
                        nc.gpsimd.tensor_scalar(
                            E[:, 2, a:b], d2, 0.0, None, OP.is_equal
                        )

                # target MOMENT lanes (class-minor), from the uint8 labels:
                #   lane 0 = 1 (memset), lane 1 = t (ACT copy, or DVE
                #   TensorCopy for balance), lane 2 = t^2 (ACT Square),
                #   lane 3 = [t == 2] (one-op tensor_scalar on Pool/DVE)
                T4 = T[:, : 4 * fd].rearrange("p (f c) -> p f c", c=4)
                if it in COPY_ON_DVE:
                    nc.vector.tensor_copy(T4[:, :, 1], tu)
                else:
                    nc.scalar.copy(T4[:, :, 1], tu)
                nc.scalar.activation(T4[:, :, 2], tu, AF.Square)
                if LANE3_MODE == "abs":
                    nc.scalar.activation(
                        T4[:, :, 3], tu, AF.Abs, bias=bias_m1, scale=1.0
                    )
                else:
                    lane3_eng = nc.vector if it in LANE3_ON_DVE else nc.gpsimd
                    lane3_eng.tensor_scalar(
                        T4[:, :, 3], tu, 2.0, None, OP.is_equal
                    )

                # 128-wide chunks: T (class-minor, contiguous) is the
                # stationary weights; E streams as the moving operand via a
                # multi-dim (f outer, c inner) AP. PSUM entry [4f'+j, 4f+c]
                # accumulates sum_p T_j * E_c; the host decodes the 4x4
                # diagonal blocks (transposed vs the E^T @ T layout).
                for w_i in range(fd // 32):
                    sl = slice(w_i * 32, (w_i + 1) * 32)
                    Ec = E[:, :, sl].rearrange("p c f -> p f c")
                    Tc = T[:, 128 * w_i : 128 * (w_i + 1)]
                    nc.tensor.matmul(
                        P_acc, Tc, Ec,
                        start=(mm == 0), stop=(mm == n_mm - 1),
                    )
                    mm += 1

        conf_sb = eht.tile([P, 128], f32, name="conf_sb")
        nc.vector.tensor_copy(conf_sb, P_acc)
        nc.sync.dma_start(out=conf, in_=conf_sb)


_NC_CACHE = {}


def _get_nc(n_reps=1):
    if n_reps in _NC_CACHE:
        return _NC_CACHE[n_reps]
    import concourse.bacc as bacc
    import concourse.mybir as mybir
    import concourse.tile as tile

    nc = bacc.Bacc(
        "TRN2",
        target_bir_lowering=False,
        debug=False,
        enable_asserts=False,
        num_devices=N_CORES,
    )
    x = nc.dram_tensor(
        "x", [B_LOC, C, H, W], mybir.dt.bfloat16, kind="ExternalInput"
    ).ap()
    t = nc.dram_tensor("t", [B_LOC, H, W], mybir.dt.uint8, kind="ExternalInput").ap()
    conf = nc.dram_tensor("conf", [P, 128], mybir.dt.float32, kind="ExternalOutput").ap()

    with tile.TileContext(nc) as tc:
        build_body(tc, {"conf": conf}, {"x": x, "t": t}, n_reps=n_reps)
    nc.compile()
    _NC_CACHE[n_reps] = nc
    return nc


# Moment basis: T-lane j holds mu_j(t); V[j, d] = mu_j(d) for class d.
# Row 3 matches LANE3_MODE ("abs" -> |t-1|, "eq2" -> [t==2]).
def _mom_v() -> np.ndarray:
    row3 = [1, 0, 1, 2] if LANE3_MODE == "abs" else [0, 0, 1, 0]
    return np.array(
        [[1, 1, 1, 1], [0, 1, 2, 3], [0, 1, 4, 9], row3], dtype=np.float64
    )


def make_in_maps(x_full: np.ndarray, t_full: np.ndarray) -> list:
    """Shard full inputs into per-core {x: bf16, t: u8} maps."""
    import ml_dtypes

    x_bf = np.asarray(x_full, dtype=np.float32).astype(ml_dtypes.bfloat16)
    t_u8 = np.asarray(t_full).astype(np.uint8)
    in_maps = []
    for ci in range(N_CORES):
        sl = slice(ci * B_LOC, (ci + 1) * B_LOC)
        in_maps.append(
            {
                "x": np.ascontiguousarray(x_bf[sl]),
                "t": np.ascontiguousarray(t_u8[sl]),
            }
        )
    return in_maps


def decode_conf(conf_sum: np.ndarray) -> np.ndarray:
    """[128,128] summed PSUM dump(s) -> moment-basis matrix M' [4,4].

    M'[c, j] = sum_pix elane_c * mu_j(t), with elane = (e0, e1, e2, 1).
    PSUM holds O[4i+j, 4i+c] (T as weights, E as moving operand), so the
    diagonal-block sum is transposed into [c, j]."""
    O = conf_sum.reshape(32, 4, 32, 4)
    return O[np.arange(32), :, np.arange(32), :].sum(axis=0).T


def finish(Mp: np.ndarray) -> np.float32:
    """Moment-basis M' [4,4] -> dice loss scalar (f32 math as the reference)."""
    mom_v = _mom_v()
    Mp = Mp.astype(np.float64)
    # rows c<3: M[c, :] (target-class histogram within pred class c)
    M_rows = np.linalg.solve(mom_v, Mp[:3, :].T).T  # [3, 4]
    M_rows = np.rint(M_rows)
    tgt = np.rint(np.linalg.solve(mom_v, Mp[3, :]))  # [4]
    n_tot = Mp[3, 0]
    pred = np.empty(4)
    pred[:3] = Mp[:3, 0]
    pred[3] = n_tot - pred[:3].sum()
    inter = np.empty(4)
    inter[:3] = np.diag(M_rows[:, :3])
    inter[3] = tgt[3] - M_rows[:, 3].sum()

    inter32 = inter.astype(np.float32)
    union32 = (pred + tgt).astype(np.float32)
    eps32 = np.float32(EPS)
    dice = (np.float32(2.0) * inter32 + eps32) / (union32 + eps32)
    losses = np.float32(1.0) - dice
    return np.float32(losses.mean(dtype=np.float32))


LAST_RESULT = None


def kernel(**inputs) -> np.ndarray:
    from concourse import bass_utils

    nc = _get_nc()
    in_maps = make_in_maps(inputs["input"], inputs["target"])

    # Transient NRT device errors (e.g. NRT_EXEC_UNIT_UNRECOVERABLE) have
    # been observed to succeed on retry in this environment.
    last_exc = None
    for attempt in range(3):
        try:
            res = bass_utils.run_bass_kernel_spmd(
                nc, in_maps, core_ids=list(range(N_CORES))
            )
            break
        except Exception as exc:  # noqa: BLE001
            last_exc = exc
            import time as _time

            _time.sleep(2.0 * (attempt + 1))
    else:
        raise last_exc
    global LAST_RESULT
    LAST_RESULT = res

    conf_sum = np.zeros((P, 128), dtype=np.float64)
    for r in res.results:
        conf_sum += r["conf"].astype(np.float64)
    Mp = decode_conf(conf_sum)
    return finish(Mp)
